# revision 14
# baseline (speedup 1.0000x reference)
"""Trainium2 Bass kernel for nn_Mlp_cnn_shift (dense CNN MLP with 3x3 patch-shift
and a softmax-gated mix of two branches).

Strategy
--------
Data-parallel over the 16 (B,T) frames: each of the 8 NeuronCores processes 2
frames end-to-end.  All activations are kept channel-major ([C, tokens]) so the
channel contraction of every matmul has K on partitions, and `x` is
pre-transposed/cast on the host so no on-device transpose is needed.

Patch-shift handling (v3 — packed dense hidden layout):
 * The 9 hid shift groups (8 x 114 channels + 112 channels) pack EXACTLY into
   8 dense 128-row blocks: block g holds shift group g (g=0..7) at rows
   0..113 and a 14-row slice of group 8 at rows 114..127.  xh is stored in
   this layout, in zero-padded token form (row pitch 57 = 56 cols + 1 zero
   pad col, 58-token zero guards per frame) so every (dh,dw) roll becomes a
   pure token offset with the reference's zero-fill boundary reproduced
   exactly.
 * fc (A phase): 8 output blocks x 4 K-blocks of x — no 9/8 padding tax.
 * fc2 (w branch, unshifted): contracts over the 8 dense K-blocks in place.
 * fc1 (h branch): per output block, 8 matmuls read group g directly at
   partitions 0..113 of block g with that group's shifted token window
   (K=114 at base partition 0 is legal), plus one K=112 matmul over a small
   gathered group-8 tile.  The gather is 8 contiguous [14 x 456] SBUF->SBUF
   DMA copies per row group (~51 KB), issued right after each block's fc
   evacuation; fc1 runs one row-group behind so the copies are off the
   critical path.
 * inverse shift (on gelu(y), C=512): interleaved-512 row permutation; the
   gelu PSUM evacuation writes each group's rows at its shifted, edge-clipped
   token positions.

The only cross-core coupling is the global (T,H,W) mean feeding the softmax
gate, done as per-batch subgroup AllReduces, one per frame; the second one's
latency window is bridged by warm matmuls that keep the PE activity monitor
from re-throttling the clock.  The gate computes s = sigmoid(l1-l0) directly
from host-precomputed difference weights (softmax over 2 streams == sigmoid
of the logit difference), then gated = h - s*(h-w).

bf16 matmuls with f32 PSUM accumulation; output stored bf16, upcast on host.
"""

import os
import sys

for _p in ("/opt/trn_rl_repo",):
    if os.path.isdir(_p) and _p not in sys.path:
        sys.path.append(_p)

import numpy as np
import ml_dtypes

import concourse.bass as bass  # noqa: F401
import concourse.mybir as mybir
import concourse.tile as tile
from concourse import bacc
from concourse.bass_utils import run_bass_kernel_spmd

# ---------------------------------------------------------------- constants
SHIFTS = [(1, 1), (1, 0), (1, -1), (0, 1), (0, 0), (0, -1), (-1, 1), (-1, 0), (-1, -1)]
NG = 9
B, T, H, W, C = 2, 8, 56, 56, 512
HID = 1024
NCORES = 8
NF = (B * T) // NCORES          # frames per core = 2
HWTOK = H * W                   # 3136 tokens per frame
RP = W + 1                      # padded row pitch = 57
GUARD = RP + 1                  # 58 zero tokens on each end
FRPAD = RP * H                  # 3192
XHSPAN = GUARD + FRPAD + GUARD  # 3308
RG = 7                          # row groups per frame
RGR = H // RG                   # 8 rows per group
RGT = RGR * W                   # 448 valid tokens per row group
RGP = RGR * RP                  # 456 padded tokens per row group
GS_HID = 114                    # hid shift-group size (8 main groups)
G8H = HID - 8 * GS_HID          # 112 channels in hid group 8
G8B = G8H // 8                  # 14 group-8 rows per dense block
GS_C = 57                       # C shift-group size (8 main groups)
G8N = C - 8 * GS_C              # 56 channels in the 9th C group
CB = C // 128                   # 4 dense C row-blocks (interleaved layout)
CCB = C // 128                  # 4
HB = HID // 128                 # 8 dense hid blocks
MEAN_N = float(T * H * W)
WARM_MMS = 128                  # AllReduce-window bridge matmuls
HEAD_MMS = 7                    # kernel-head HAM warmup matmuls
DCK = 384                       # output-phase token chunk

F32 = mybir.dt.float32
BF16 = mybir.dt.bfloat16
BF16_NP = ml_dtypes.bfloat16

_CACHE = {}


def _qof():
    """Interleaved-512 layout: padded row q -> real channel (a permutation).

    Main group g (57 ch) at rows 64g..64g+57; g8 channel i at row
    64*(i//7) + 57 + (i%7).
    """
    q = np.full((C,), -1, np.int64)
    for g in range(8):
        q[64 * g:64 * g + GS_C] = np.arange(GS_C * g, GS_C * (g + 1))
    for i in range(G8N):
        q[64 * (i // 7) + GS_C + (i % 7)] = 8 * GS_C + i
    assert (np.sort(q) == np.arange(C)).all()
    return q


def _hperm():
    """Packed dense hid layout: row 128g+i -> channel 114g+i (i<114), row
    128g+114+j -> group-8 channel 14g+j (j<14)."""
    p = np.empty((HID,), np.int64)
    for g in range(HB):
        p[128 * g:128 * g + GS_HID] = np.arange(GS_HID * g, GS_HID * (g + 1))
        p[128 * g + GS_HID:128 * (g + 1)] = 8 * GS_HID + np.arange(
            G8B * g, G8B * (g + 1))
    assert (np.sort(p) == np.arange(HID)).all()
    return p


# ---------------------------------------------------------------- device kernel
def build_nc():
    nc = bacc.Bacc("TRN2", target_bir_lowering=False, debug=False, num_devices=NCORES)

    dp = nc.declare_dram_parameter
    xT = dp("xT", [NF, 128, CCB, HWTOK], BF16, isOutput=False)
    fcw = dp("fcw", [128, CCB, HID], BF16, isOutput=False)
    fcb = dp("fcb", [128, HB], F32, isOutput=False)
    fc1wd = dp("fc1wd", [128, HB, C], BF16, isOutput=False)
    fc1w8 = dp("fc1w8", [128, C], BF16, isOutput=False)
    fc1b = dp("fc1b", [128, CB], F32, isOutput=False)
    fc2w = dp("fc2w", [128, HB, C], BF16, isOutput=False)
    fc2b = dp("fc2b", [128, CB], F32, isOutput=False)
    projw = dp("projw", [128, CB, C], BF16, isOutput=False)
    projb = dp("projb", [128, C], F32, isOutput=False)
    rw1w = dp("rw1w", [128, CB, 128], BF16, isOutput=False)
    rw1b = dp("rw1b", [128, 1], F32, isOutput=False)
    rwdw = dp("rwdw", [128, C], BF16, isOutput=False)
    rwdb = dp("rwdb", [128, CB], F32, isOutput=False)
    out_d = dp("out", [NF, HWTOK, C], BF16, isOutput=True)

    # spill space for d = h - w of each frame + collective bounce buffers
    dsp = [nc.dram_tensor(f"dsp{f}", [128, CB, HWTOK], BF16) for f in range(NF)]
    ccin = [nc.dram_tensor(f"ccin{f}", [128, CB], F32) for f in range(NF)]
    ccout = [nc.dram_tensor(f"ccout{f}", [128, CB], F32) for f in range(NF)]

    AF = mybir.ActivationFunctionType
    ALU = mybir.AluOpType
    GROUPS = [list(range(NCORES // 2)), list(range(NCORES // 2, NCORES))]

    with tile.TileContext(nc, num_cores=NCORES) as tc:
        with (
            tc.tile_pool(name="singles", bufs=1) as singles,
            tc.tile_pool(name="xh_pool", bufs=1) as xh_pool,
            tc.tile_pool(name="h_pool", bufs=2) as h_pool,
            tc.tile_pool(name="w_pool", bufs=3) as w_pool,
            tc.tile_pool(name="xt_pool", bufs=3) as xt_pool,
            tc.tile_pool(name="ostage", bufs=6) as ostage,
            tc.tile_pool(name="dstream", bufs=5) as dstream,
            tc.tile_pool(name="small", bufs=1) as small,
            tc.tile_pool(name="mmpsum", bufs=8, space="PSUM") as mmpsum,
        ):
            # ---- first input slices + weights stream in from the very top
            # so the first real matmul can start ~2us in; a short junk-matmul
            # run keeps the HAM activity window busy meanwhile.  xt slices
            # are prefetched one row-group ahead throughout.
            xts = {}

            def xt_fetch(f, rg):
                if (f, rg) not in xts and f < NF and rg < RG:
                    t = xt_pool.tile([128, CCB, RGT], BF16, tag="xt")
                    nc.sync.dma_start(
                        out=t, in_=xT[f, :, :, rg * RGT:(rg + 1) * RGT]
                    )
                    xts[(f, rg)] = t

            xt_fetch(0, 0)

            fcw_s = singles.tile([128, CCB, HID], BF16, name="fcw_s")
            for k in range(CCB):
                nc.sync.dma_start(out=fcw_s[:, k, :], in_=fcw[:, k, :])

            def load(name, shape, dtype, src):
                t = singles.tile(shape, dtype, name=name)
                nc.sync.dma_start(out=t, in_=src[:])
                return t

            fcb_s = load("fcb_s", [128, HB], F32, fcb)
            xt_fetch(0, 1)
            fc2w_s = load("fc2w_s", [128, HB, C], BF16, fc2w)
            fc2b_s = load("fc2b_s", [128, CB], F32, fc2b)
            fc1wd_s = load("fc1wd_s", [128, HB, C], BF16, fc1wd)
            fc1w8_s = load("fc1w8_s", [128, C], BF16, fc1w8)
            fc1b_s = load("fc1b_s", [128, CB], F32, fc1b)

            _rest = {}

            def load_rest():
                _rest["projw_s"] = load("projw_s", [128, CB, C], BF16, projw)
                _rest["projb_s"] = load("projb_s", [128, C], F32, projb)
                _rest["rw1w_s"] = load("rw1w_s", [128, CB, 128], BF16, rw1w)
                _rest["rw1b_s"] = load("rw1b_s", [128, 1], F32, rw1b)
                _rest["rwdw_s"] = load("rwdw_s", [128, C], BF16, rwdw)
                _rest["rwdb_s"] = load("rwdb_s", [128, CB], F32, rwdb)

            # ---- activation-table prewarm (Gelu + Sigmoid LUT loads are
            # ~1.3us each; run them on a dedicated tile while DMAs stream so
            # they don't serialize with the junk matmuls' operand tile).
            pw = small.tile([128, 2], BF16, tag="pw")
            nc.vector.memset(pw, 0.0)
            nc.scalar.activation(out=pw[:, 0:1], in_=pw[:, 0:1], func=AF.Gelu)
            nc.scalar.activation(out=pw[:, 1:2], in_=pw[:, 1:2], func=AF.Sigmoid)

            jt = singles.tile([128, 640], BF16, name="jt")
            nc.vector.memset(jt[:, 0:128], 0.0)
            nc.vector.memset(jt[:, 128:640], 0.0)
            for wi in range(HEAD_MMS):
                wp = mmpsum.tile([128, 512], F32, tag="mm", name=f"hw{wi}")
                nc.tensor.matmul(
                    wp[:, :512], lhsT=jt[:, 0:128], rhs=jt[:, 128:640],
                    start=True, stop=True,
                )

            s_gate = singles.tile([128, CB], F32)   # sigmoid(l1-l0) = 1 - a0

            # xh: packed dense blocks, padded token layout, persistent.
            xh = xh_pool.tile([128, HB, XHSPAN], BF16)
            nc.vector.memset(xh[:, :, :GUARD], 0.0)
            nc.vector.memset(xh[:, :, GUARD + FRPAD:], 0.0)
            xh_rows = xh[:, :, GUARD:GUARD + FRPAD].rearrange(
                "p g (r c) -> p g r c", c=RP
            )
            nc.vector.memset(xh_rows[:, :, :, W:], 0.0)
            # gathered group-8 tile (copies bring zeroed pad cols with them)
            xh8 = xh_pool.tile([128, XHSPAN], BF16, name="xh8")
            nc.vector.memset(xh8[:, :GUARD], 0.0)
            nc.vector.memset(xh8[:, GUARD + FRPAD:], 0.0)

            hw_tiles = []

            def shifted_rhs(g, rg):
                sh, sw = SHIFTS[g]
                off = -(sh * RP + sw)
                s0 = GUARD + rg * RGP + off
                return xh[0:GS_HID, g, s0:s0 + RGP].rearrange(
                    "p (r c) -> p r c", c=RP
                )[:, :, :W]

            def shifted_rhs8(rg):
                sh, sw = SHIFTS[8]
                off = -(sh * RP + sw)
                s0 = GUARD + rg * RGP + off
                return xh8[0:G8H, s0:s0 + RGP].rearrange(
                    "p (r c) -> p r c", c=RP
                )[:, :, :W]

            for f in range(NF):
                h_t = h_pool.tile([128, CB, HWTOK], BF16, tag="h")
                nc.gpsimd.memset(h_t[:], 0.0)
                h4 = h_t.rearrange("p c (i j) -> p c i j", j=W)
                wsum_st = small.tile([128, CB, RG], F32, tag=f"wsst{f}")
                dsum_st = small.tile([128, CB, RG], F32, tag=f"dsst{f}")

                def h_evac(ps, rg, mb):
                    """Inverse-shift evacuation of one 128-row block: two
                    57-row main-group writes, plus the two 7-row g8 fragments
                    via a 32-aligned scratch activation (compute-engine APs
                    must start at 32-aligned partitions) whose g8 rows are
                    then DMA-copied into place."""
                    ps3 = ps[:, :RGT].rearrange("p (r c) -> p r c", c=W)
                    for half in range(2):
                        p0 = half * 64
                        g = 2 * mb + half
                        sh, sw = SHIFTS[g]
                        i0 = max(0, 8 * rg - sh)
                        i1 = min(H, 8 * rg + 8 - sh)
                        j0, j1 = max(0, -sw), min(W, W - sw)
                        nc.scalar.activation(
                            out=h4[p0:p0 + GS_C, mb, i0:i1, j0:j1],
                            in_=ps3[
                                p0:p0 + GS_C,
                                i0 + sh - 8 * rg:i1 + sh - 8 * rg,
                                j0 + sw:j1 + sw,
                            ],
                            func=AF.Gelu,
                            bias=fc1b_s[p0:p0 + GS_C, mb:mb + 1],
                        )
                    # g8 fragments via 32-row scratch activations (a pattern
                    # starting at partition 32k may span at most 32
                    # partitions, so one per half)
                    sh, sw = SHIFTS[8]
                    i0 = max(0, 8 * rg - sh)
                    i1 = min(H, 8 * rg + 8 - sh)
                    j0, j1 = max(0, -sw), min(W, W - sw)
                    scr = w_pool.tile([128, RGR + 1, W], BF16, tag="scr")
                    for half in range(2):
                        q0 = half * 64 + 32
                        nc.scalar.activation(
                            out=scr[q0:q0 + 32, 0:i1 - i0, j0:j1],
                            in_=ps3[
                                q0:q0 + 32,
                                i0 + sh - 8 * rg:i1 + sh - 8 * rg,
                                j0 + sw:j1 + sw,
                            ],
                            func=AF.Gelu,
                            bias=fc1b_s[q0:q0 + 32, mb:mb + 1],
                        )
                        f0 = half * 64 + GS_C
                        nc.sync.dma_start(
                            out=h4[f0:f0 + 7, mb, i0:i1, j0:j1],
                            in_=scr[f0:f0 + 7, 0:i1 - i0, j0:j1],
                        )

                prev_w = [None] * RG

                def d_spill(rg):
                    w_prev = prev_w[rg]
                    nc.vector.tensor_tensor(
                        w_prev[:],
                        h_t[:, :, rg * RGT:(rg + 1) * RGT],
                        w_prev[:],
                        ALU.subtract,
                    )
                    nc.sync.dma_start(
                        out=dsp[f][:, :, rg * RGT:(rg + 1) * RGT], in_=w_prev[:]
                    )
                    # gate-sum bookkeeping: sum(h) = sum(d) + sum(w)
                    nc.vector.tensor_reduce(
                        out=dsum_st[:, :, rg:rg + 1], in_=w_prev[:],
                        axis=mybir.AxisListType.X, op=ALU.add,
                    )

                def fc1_rg(rg):
                    for mb in range(CB):
                        ps = mmpsum.tile([128, 512], F32, tag="mm")
                        for g in range(HB):
                            nc.tensor.matmul(
                                ps[:, :RGT],
                                lhsT=fc1wd_s[0:GS_HID, g, mb * 128:(mb + 1) * 128],
                                rhs=shifted_rhs(g, rg),
                                start=(g == 0),
                                stop=False,
                            )
                        nc.tensor.matmul(
                            ps[:, :RGT],
                            lhsT=fc1w8_s[0:G8H, mb * 128:(mb + 1) * 128],
                            rhs=shifted_rhs8(rg),
                            start=False,
                            stop=True,
                        )
                        h_evac(ps, rg, mb)

                for rg in range(RG):
                    # ---------- A: xh = gelu(x @ fc_w + fc_b), packed dense
                    xt_fetch(f, rg)
                    if rg + 1 < RG:
                        xt_fetch(f, rg + 1)
                    else:
                        xt_fetch(f + 1, 0)
                    xt_t = xts.pop((f, rg))
                    base = GUARD + rg * RGP
                    for mb in range(HB):
                        ps = mmpsum.tile([128, 512], F32, tag="mm")
                        for k in range(CCB):
                            nc.tensor.matmul(
                                ps[:, :RGT],
                                lhsT=fcw_s[:, k, mb * 128:(mb + 1) * 128],
                                rhs=xt_t[:, k, :],
                                start=(k == 0),
                                stop=(k == CCB - 1),
                            )
                        dst = xh[:, mb, base:base + RGP].rearrange(
                            "p (r c) -> p r c", c=RP
                        )[:, :, :W]
                        src = ps[:, :RGT].rearrange("p (r c) -> p r c", c=W)
                        nc.scalar.activation(
                            out=dst, in_=src, func=AF.Gelu,
                            bias=fcb_s[:, mb:mb + 1],
                        )
                        # gather this block's group-8 slice (contiguous span,
                        # pad cols are zero in both src and dst)
                        nc.sync.dma_start(
                            out=xh8[G8B * mb:G8B * (mb + 1), base:base + RGP],
                            in_=xh[GS_HID:128, mb, base:base + RGP],
                        )

                    # ---------- B: w = gelu(xh @ fc2_w + b), dense contraction
                    rhs_pl = xh[:, :, base:base + RGP].rearrange(
                        "p g (r c) -> p g r c", c=RP)[:, :, :, :W]
                    w_rg = w_pool.tile([128, CB, RGT], BF16, tag="wrg", bufs=5)
                    for mb in range(CB):
                        ps = mmpsum.tile([128, 512], F32, tag="mm")
                        for kb in range(HB):
                            nc.tensor.matmul(
                                ps[:, :RGT],
                                lhsT=fc2w_s[:, kb, mb * 128:(mb + 1) * 128],
                                rhs=rhs_pl[:, kb],
                                start=(kb == 0),
                                stop=(kb == HB - 1),
                            )
                        dst = w_rg[:, mb, :].rearrange("p (r c) -> p r c", c=W)
                        srcp = ps[:, :RGT].rearrange("p (r c) -> p r c", c=W)
                        nc.scalar.activation(
                            out=dst, in_=srcp, func=AF.Gelu,
                            bias=fc2b_s[:, mb:mb + 1],
                        )
                    nc.vector.tensor_reduce(
                        out=wsum_st[:, :, rg:rg + 1], in_=w_rg[:],
                        axis=mybir.AxisListType.X, op=ALU.add,
                    )
                    prev_w[rg] = w_rg

                    if f == 0 and rg == 0:
                        load_rest()
                        projw_s = _rest["projw_s"]; projb_s = _rest["projb_s"]
                        rw1w_s = _rest["rw1w_s"]; rw1b_s = _rest["rw1b_s"]
                        rwdw_s = _rest["rwdw_s"]; rwdb_s = _rest["rwdb_s"]

                    # ---------- C: h (lag 2 so the group-8 gather copies
                    # always have a full row-group step of slack)
                    if rg >= 2:
                        fc1_rg(rg - 2)
                        if rg >= 3:
                            d_spill(rg - 3)
                # gate partial sums: everything except the last row-group's
                # d-sum is folded into `pbase` before d_spill(6), so the
                # AllReduce launch chain after the last matmul is just one
                # reduce + one add + the ccin DMA.
                ws = small.tile([128, CB], F32, tag=f"ws{f}")
                nc.vector.tensor_reduce(
                    out=ws, in_=wsum_st[:], axis=mybir.AxisListType.X, op=ALU.add
                )
                fc1_rg(RG - 2)
                d_spill(RG - 3)
                fc1_rg(RG - 1)
                d_spill(RG - 2)
                dsm = small.tile([128, CB], F32, tag=f"dsm{f}")
                nc.vector.tensor_reduce(
                    out=dsm, in_=dsum_st[:, :, 0:RG - 1],
                    axis=mybir.AxisListType.X, op=ALU.add,
                )
                pbase = small.tile([128, CB], F32, tag=f"pb{f}")
                nc.vector.tensor_scalar_mul(pbase, ws, 2.0)
                nc.vector.tensor_tensor(pbase, pbase, dsm, ALU.add)
                d_spill(RG - 1)

                # part = sum(h) + sum(w) = sum(d) + 2*sum(w)
                part = small.tile([128, CB], F32, tag=f"part{f}")
                nc.vector.tensor_tensor(
                    part, pbase,
                    dsum_st[:, :, RG - 1:RG].rearrange("p c o -> p (c o)"),
                    ALU.add,
                )
                nc.sync.dma_start(out=ccin[f][:], in_=part)
                nc.gpsimd.collective_compute(
                    "AllReduce",
                    ALU.add,
                    replica_groups=GROUPS,
                    ins=[ccin[f][:]],
                    outs=[ccout[f][:]],
                )

                hw_tiles.append(h_t)

            # bridge the second AllReduce's latency window (collective floor
            # + cross-core launch skew) with junk matmuls so the PE activity
            # monitor keeps the full clock.
            for wi in range(WARM_MMS):
                wp = mmpsum.tile([128, 512], F32, tag="mm", name=f"warm{wi}")
                nc.tensor.matmul(
                    wp[:, :512],
                    lhsT=fcw_s[:, 0, 0:128],
                    rhs=fcw_s[:, 1, 0:512],
                    start=True,
                    stop=True,
                )

            # ---------------- gate: s = sigmoid(l1 - l0) = 1 - a0
            # (1/MEAN_N folded into rw1w, l1-l0 weights precomputed on host)
            za = small.tile([128, 2 * CB], F32, tag="za")
            nc.sync.dma_start(out=za[:, 0:CB], in_=ccout[0][:])
            nc.sync.dma_start(out=za[:, CB:2 * CB], in_=ccout[1][:])
            zbf = small.tile([128, CB], BF16, tag="zbf")
            nc.vector.tensor_tensor(zbf, za[:, 0:CB], za[:, CB:2 * CB], ALU.add)

            psg = mmpsum.tile([128, 512], F32, tag="mm", name="psg")[:, :1]
            for k in range(CB):
                nc.tensor.matmul(
                    psg,
                    lhsT=rw1w_s[:, k, :],
                    rhs=zbf[:, k:k + 1],
                    start=(k == 0),
                    stop=(k == CB - 1),
                )
            gv = small.tile([128, 1], BF16, tag="gv")
            nc.scalar.activation(out=gv, in_=psg, func=AF.Gelu, bias=rw1b_s[:, 0:1])
            psu = mmpsum.tile([128, 512], F32, tag="mm", name="psu")[:, :CB]
            for m in range(CB):
                nc.tensor.matmul(
                    psu[:, m:m + 1],
                    lhsT=rwdw_s[:, m * 128:(m + 1) * 128],
                    rhs=gv,
                    start=True,
                    stop=True,
                )
            ldif = small.tile([128, CB], F32, tag="ldif")
            nc.vector.tensor_tensor(ldif, psu, rwdb_s, ALU.add)
            nc.scalar.activation(out=s_gate, in_=ldif, func=AF.Sigmoid)

            # ---------------- D: out = (h - s*d) @ proj_w + proj_b
            def emit_out(pp, fidx, t0, M):
                ot = ostage.tile([128, C], BF16, tag="ot")
                nc.vector.tensor_tensor(ot[:M], pp[:M, :C], projb_s[:M], ALU.add)
                nc.sync.dma_start(out=out_d[fidx, t0:t0 + M, :], in_=ot[:M])

            # h is resident for both frames; stream each frame's d back in
            # DCK-token chunks, gate into the d tile, then project.  The two
            # frames' chunks are interleaved so their independent
            # DMA->gate->matmul chains hide each other's latency; scale and
            # subtract alternate per k-block (ScalarE / DVE) so the first
            # proj matmul starts as soon as k-block 0 is gated.
            for ck0 in range(0, HWTOK, DCK):
                for fidx in (0, 1):
                    h_t = hw_tiles[fidx]
                    CK = min(DCK, HWTOK - ck0)
                    dc = dstream.tile([128, CB, DCK], BF16, tag="wc")
                    nc.sync.dma_start(
                        out=dc[:, :, :CK], in_=dsp[fidx][:, :, ck0:ck0 + CK]
                    )
                    for kb in range(CB):
                        if kb < 3:
                            nc.scalar.activation(
                                out=dc[:, kb, :CK], in_=dc[:, kb, :CK],
                                func=AF.Copy, scale=s_gate[:, kb:kb + 1],
                            )
                        else:
                            nc.vector.tensor_scalar_mul(
                                dc[:, kb, :CK], dc[:, kb, :CK],
                                s_gate[:, kb:kb + 1],
                            )
                        nc.vector.tensor_tensor(
                            dc[:, kb, :CK],
                            h_t[:, kb, ck0:ck0 + CK],
                            dc[:, kb, :CK],
                            ALU.subtract,
                        )
                    m0 = 0
                    while m0 < CK:
                        M = min(128, CK - m0)
                        pp = mmpsum.tile([128, 512], F32, tag="mm")
                        for kb in range(CB):
                            nc.tensor.matmul(
                                pp[:M, :C],
                                lhsT=dc[:, kb, m0:m0 + M],
                                rhs=projw_s[:, kb, :],
                                start=(kb == 0),
                                stop=(kb == CB - 1),
                            )
                        emit_out(pp, fidx, ck0 + m0, M)
                        m0 += M

    nc.compile()
    return nc


# ---------------------------------------------------------------- host side
def _prep_weights(fc_w, fc_b, fc1_w, fc1_b, fc2_w, fc2_b,
                  rw1_w, rw1_b, rw2_w, rw2_b, proj_w, proj_b):
    f32 = np.float32
    qof = _qof()
    hp = _hperm()

    # fc: columns in packed dense hid order, 4 k-blocks of x on partitions
    fcw_h = np.ascontiguousarray(
        fc_w[:, hp].reshape(CCB, 128, HID).transpose(1, 0, 2)
    ).astype(BF16_NP)
    fcb_h = np.ascontiguousarray(fc_b[hp].reshape(HB, 128).T).astype(f32)

    # fc1/fc2: rows in packed dense order, columns in interleaved-512 order
    fc1wd_h = np.ascontiguousarray(
        fc1_w[hp][:, qof].reshape(HB, 128, C).transpose(1, 0, 2)
    ).astype(BF16_NP)
    fc2w_h = np.ascontiguousarray(
        fc2_w[hp][:, qof].reshape(HB, 128, C).transpose(1, 0, 2)
    ).astype(BF16_NP)
    w8 = np.zeros((128, C), f32)
    w8[:G8H] = fc1_w[8 * GS_HID:][:, qof]
    fc1w8_h = np.ascontiguousarray(w8).astype(BF16_NP)

    fc1b_h = np.ascontiguousarray(fc1_b[qof].reshape(CB, 128).T).astype(f32)
    fc2b_h = np.ascontiguousarray(fc2_b[qof].reshape(CB, 128).T).astype(f32)

    projw_h = np.ascontiguousarray(
        proj_w[qof].reshape(CB, 128, C).transpose(1, 0, 2)
    ).astype(BF16_NP)
    projb_h = np.ascontiguousarray(
        np.broadcast_to(proj_b[None, :], (128, C))
    ).astype(f32)

    rw1w_h = np.ascontiguousarray(
        (rw1_w / MEAN_N)[qof].reshape(CB, 128, C // 4).transpose(1, 0, 2)
    ).astype(BF16_NP)
    rw1b_h = np.ascontiguousarray(rw1_b[:, None]).astype(f32)

    # gate difference weights: s = sigmoid(l1 - l0) = 1 - a0, so
    # gated = h*a0 + w*(1-a0) = h - s*(h-w)
    rwdw_h = np.ascontiguousarray(
        rw2_w[:, 2 * qof + 1] - rw2_w[:, 2 * qof]
    ).astype(BF16_NP)
    rwdb_h = np.ascontiguousarray(
        (rw2_b[2 * qof + 1] - rw2_b[2 * qof]).reshape(CB, 128).T
    ).astype(f32)

    return dict(
        fcw=fcw_h, fcb=fcb_h, fc1wd=fc1wd_h, fc1w8=fc1w8_h, fc1b=fc1b_h,
        fc2w=fc2w_h, fc2b=fc2b_h, projw=projw_h, projb=projb_h,
        rw1w=rw1w_h, rw1b=rw1b_h, rwdw=rwdw_h, rwdb=rwdb_h,
    )


def _get_nc():
    if "nc" not in _CACHE:
        _CACHE["nc"] = build_nc()
    return _CACHE["nc"]


def run(inputs, trace=False, trace_kwargs=None):
    """Run the SPMD kernel; returns (full_output, BassKernelResults)."""
    x = np.asarray(inputs["x"], np.float32)
    shared = _prep_weights(
        np.asarray(inputs["fc_w"], np.float32), np.asarray(inputs["fc_b"], np.float32),
        np.asarray(inputs["fc1_w"], np.float32), np.asarray(inputs["fc1_b"], np.float32),
        np.asarray(inputs["fc2_w"], np.float32), np.asarray(inputs["fc2_b"], np.float32),
        np.asarray(inputs["rw1_w"], np.float32), np.asarray(inputs["rw1_b"], np.float32),
        np.asarray(inputs["rw2_w"], np.float32), np.asarray(inputs["rw2_b"], np.float32),
        np.asarray(inputs["proj_w"], np.float32), np.asarray(inputs["proj_b"], np.float32),
    )

    xf = x.reshape(B * T, HWTOK, C)
    in_maps = []
    for c in range(NCORES):
        sh = xf[NF * c:NF * (c + 1)]                      # [NF, 3136, 512]
        xt = sh.transpose(0, 2, 1).reshape(NF, CCB, 128, HWTOK)
        xt = np.ascontiguousarray(xt.transpose(0, 2, 1, 3)).astype(BF16_NP)
        m = dict(shared)
        m["xT"] = xt
        in_maps.append(m)

    nc = _get_nc()
    res = run_bass_kernel_spmd(
        nc, in_maps, list(range(NCORES)),
        trace=trace, **(dict(trace_kwargs=trace_kwargs) if trace_kwargs else {}),
    )

    out = np.empty((B * T, HWTOK, C), np.float32)
    for c in range(NCORES):
        out[NF * c:NF * (c + 1)] = np.asarray(
            res.results[c]["out"], dtype=np.float32
        )
    return out.reshape(B, T, H, W, C), res


def kernel(**inputs) -> np.ndarray:
    full, _ = run(inputs, trace=False)
    return full


# revision 20
# speedup vs baseline: 1.0238x; 1.0238x over previous
"""Trainium2 Bass kernel for nn_Mlp_cnn_shift (dense CNN MLP with 3x3 patch-shift
and a softmax-gated mix of two branches).

Strategy
--------
Data-parallel over the 16 (B,T) frames: each of the 8 NeuronCores processes 2
frames end-to-end.  All activations are kept channel-major ([C, tokens]) so the
channel contraction of every matmul has K on partitions, and `x` is
pre-transposed/cast on the host so no on-device transpose is needed.

Patch-shift handling (v3 — packed dense hidden layout):
 * The 9 hid shift groups (8 x 114 channels + 112 channels) pack EXACTLY into
   8 dense 128-row blocks: block g holds shift group g (g=0..7) at rows
   0..113 and a 14-row slice of group 8 at rows 114..127.  xh is stored in
   this layout, in zero-padded token form (row pitch 57 = 56 cols + 1 zero
   pad col, 58-token zero guards per frame) so every (dh,dw) roll becomes a
   pure token offset with the reference's zero-fill boundary reproduced
   exactly.
 * fc (A phase): 8 output blocks x 4 K-blocks of x — no 9/8 padding tax.
 * fc2 (w branch, unshifted): contracts over the 8 dense K-blocks in place.
 * fc1 (h branch): per output block, 8 matmuls read group g directly at
   partitions 0..113 of block g with that group's shifted token window
   (K=114 at base partition 0 is legal), plus one K=112 matmul over a small
   gathered group-8 tile.  The gather is 8 contiguous [14 x 456] SBUF->SBUF
   DMA copies per row group (~51 KB), issued right after each block's fc
   evacuation; fc1 runs one row-group behind so the copies are off the
   critical path.
 * inverse shift (on gelu(y), C=512): interleaved-512 row permutation; the
   gelu PSUM evacuation writes each group's rows at its shifted, edge-clipped
   token positions.

The only cross-core coupling is the global (T,H,W) mean feeding the softmax
gate, done as per-batch subgroup AllReduces, one per frame; the second one's
latency window is bridged by warm matmuls that keep the PE activity monitor
from re-throttling the clock.  The gate computes s = sigmoid(l1-l0) directly
from host-precomputed difference weights (softmax over 2 streams == sigmoid
of the logit difference), then gated = h - s*(h-w).

bf16 matmuls with f32 PSUM accumulation; output stored bf16, upcast on host.
"""

import os
import sys

for _p in ("/opt/trn_rl_repo",):
    if os.path.isdir(_p) and _p not in sys.path:
        sys.path.append(_p)

import numpy as np
import ml_dtypes

import concourse.bass as bass  # noqa: F401
import concourse.mybir as mybir
import concourse.tile as tile
from concourse import bacc
from concourse.bass_utils import run_bass_kernel_spmd

# ---------------------------------------------------------------- constants
SHIFTS = [(1, 1), (1, 0), (1, -1), (0, 1), (0, 0), (0, -1), (-1, 1), (-1, 0), (-1, -1)]
NG = 9
B, T, H, W, C = 2, 8, 56, 56, 512
HID = 1024
NCORES = 8
NF = (B * T) // NCORES          # frames per core = 2
HWTOK = H * W                   # 3136 tokens per frame
RP = W + 1                      # padded row pitch = 57
GUARD = RP + 1                  # 58 zero tokens on each end
FRPAD = RP * H                  # 3192
XHSPAN = GUARD + FRPAD + GUARD  # 3308
RG = 7                          # row groups per frame
RGR = H // RG                   # 8 rows per group
RGT = RGR * W                   # 448 valid tokens per row group
RGP = RGR * RP                  # 456 padded tokens per row group
GS_HID = 114                    # hid shift-group size (8 main groups)
G8H = HID - 8 * GS_HID          # 112 channels in hid group 8
G8B = G8H // 8                  # 14 group-8 rows per dense block
GS_C = 57                       # C shift-group size (8 main groups)
G8N = C - 8 * GS_C              # 56 channels in the 9th C group
CB = C // 128                   # 4 dense C row-blocks (interleaved layout)
CCB = C // 128                  # 4
HB = HID // 128                 # 8 dense hid blocks
MEAN_N = float(T * H * W)
WARM_MMS = 118                  # AllReduce-window bridge matmuls
HEAD_MMS = 7                    # kernel-head HAM warmup matmuls
DCK = 384                       # output-phase token chunk

F32 = mybir.dt.float32
BF16 = mybir.dt.bfloat16
BF16_NP = ml_dtypes.bfloat16

_CACHE = {}


def _qof():
    """Interleaved-512 layout: padded row q -> real channel (a permutation).

    Main group g (57 ch) at rows 64g..64g+57; g8 channel i at row
    64*(i//7) + 57 + (i%7).
    """
    q = np.full((C,), -1, np.int64)
    for g in range(8):
        q[64 * g:64 * g + GS_C] = np.arange(GS_C * g, GS_C * (g + 1))
    for i in range(G8N):
        q[64 * (i // 7) + GS_C + (i % 7)] = 8 * GS_C + i
    assert (np.sort(q) == np.arange(C)).all()
    return q


def _hperm():
    """Packed dense hid layout: row 128g+i -> channel 114g+i (i<114), row
    128g+114+j -> group-8 channel 14g+j (j<14)."""
    p = np.empty((HID,), np.int64)
    for g in range(HB):
        p[128 * g:128 * g + GS_HID] = np.arange(GS_HID * g, GS_HID * (g + 1))
        p[128 * g + GS_HID:128 * (g + 1)] = 8 * GS_HID + np.arange(
            G8B * g, G8B * (g + 1))
    assert (np.sort(p) == np.arange(HID)).all()
    return p


# ---------------------------------------------------------------- device kernel
def build_nc():
    nc = bacc.Bacc("TRN2", target_bir_lowering=False, debug=False, num_devices=NCORES)

    dp = nc.declare_dram_parameter
    xT = dp("xT", [NF, 128, CCB, HWTOK], BF16, isOutput=False)
    fcw = dp("fcw", [128, CCB, HID], BF16, isOutput=False)
    fcb = dp("fcb", [128, HB], F32, isOutput=False)
    fc1wd = dp("fc1wd", [128, HB, C], BF16, isOutput=False)
    fc1w8 = dp("fc1w8", [128, C], BF16, isOutput=False)
    fc1b = dp("fc1b", [128, CB], F32, isOutput=False)
    fc2w = dp("fc2w", [128, HB, C], BF16, isOutput=False)
    fc2b = dp("fc2b", [128, CB], F32, isOutput=False)
    projw = dp("projw", [128, CB, C], BF16, isOutput=False)
    projb = dp("projb", [128, C], F32, isOutput=False)
    rw1w = dp("rw1w", [128, CB, 128], BF16, isOutput=False)
    rw1b = dp("rw1b", [128, 1], F32, isOutput=False)
    rwdw = dp("rwdw", [128, C], BF16, isOutput=False)
    rwdb = dp("rwdb", [128, CB], F32, isOutput=False)
    out_d = dp("out", [NF, HWTOK, C], BF16, isOutput=True)

    # spill space for d = h - w of each frame + collective bounce buffers
    dsp = [nc.dram_tensor(f"dsp{f}", [128, CB, HWTOK], BF16) for f in range(NF)]
    ccin = [nc.dram_tensor(f"ccin{f}", [128, CB], F32) for f in range(NF)]
    ccout = [nc.dram_tensor(f"ccout{f}", [128, CB], F32) for f in range(NF)]

    AF = mybir.ActivationFunctionType
    ALU = mybir.AluOpType
    GROUPS = [list(range(NCORES // 2)), list(range(NCORES // 2, NCORES))]

    with tile.TileContext(nc, num_cores=NCORES) as tc:
        with (
            tc.tile_pool(name="singles", bufs=1) as singles,
            tc.tile_pool(name="xh_pool", bufs=1) as xh_pool,
            tc.tile_pool(name="h_pool", bufs=2) as h_pool,
            tc.tile_pool(name="w_pool", bufs=3) as w_pool,
            tc.tile_pool(name="xt_pool", bufs=3) as xt_pool,
            tc.tile_pool(name="ostage", bufs=6) as ostage,
            tc.tile_pool(name="dstream", bufs=5) as dstream,
            tc.tile_pool(name="small", bufs=1) as small,
            tc.tile_pool(name="mmpsum", bufs=8, space="PSUM") as mmpsum,
        ):
            # ---- first input slices + weights stream in from the very top
            # so the first real matmul can start ~2us in; a short junk-matmul
            # run keeps the HAM activity window busy meanwhile.  xt slices
            # are prefetched one row-group ahead throughout.
            xts = {}

            def xt_fetch(f, rg):
                if (f, rg) not in xts and f < NF and rg < RG:
                    t = xt_pool.tile([128, CCB, RGT], BF16, tag="xt")
                    nc.sync.dma_start(
                        out=t, in_=xT[f, :, :, rg * RGT:(rg + 1) * RGT]
                    )
                    xts[(f, rg)] = t

            xt_fetch(0, 0)

            fcw_s = singles.tile([128, CCB, HID], BF16, name="fcw_s")
            for k in range(CCB):
                nc.sync.dma_start(out=fcw_s[:, k, :], in_=fcw[:, k, :])

            def load(name, shape, dtype, src):
                t = singles.tile(shape, dtype, name=name)
                nc.sync.dma_start(out=t, in_=src[:])
                return t

            fcb_s = load("fcb_s", [128, HB], F32, fcb)
            xt_fetch(0, 1)
            fc2w_s = load("fc2w_s", [128, HB, C], BF16, fc2w)
            fc2b_s = load("fc2b_s", [128, CB], F32, fc2b)
            fc1wd_s = load("fc1wd_s", [128, HB, C], BF16, fc1wd)
            fc1w8_s = load("fc1w8_s", [128, C], BF16, fc1w8)
            fc1b_s = load("fc1b_s", [128, CB], F32, fc1b)

            _rest = {}

            def load_rest():
                _rest["projw_s"] = load("projw_s", [128, CB, C], BF16, projw)
                _rest["projb_s"] = load("projb_s", [128, C], F32, projb)
                _rest["rw1w_s"] = load("rw1w_s", [128, CB, 128], BF16, rw1w)
                _rest["rw1b_s"] = load("rw1b_s", [128, 1], F32, rw1b)
                _rest["rwdw_s"] = load("rwdw_s", [128, C], BF16, rwdw)
                _rest["rwdb_s"] = load("rwdb_s", [128, CB], F32, rwdb)

            # ---- activation-table prewarm (Gelu + Sigmoid LUT loads are
            # ~1.3us each; run them on a dedicated tile while DMAs stream so
            # they don't serialize with the junk matmuls' operand tile).
            pw = small.tile([128, 2], BF16, tag="pw")
            nc.vector.memset(pw, 0.0)
            nc.scalar.activation(out=pw[:, 0:1], in_=pw[:, 0:1], func=AF.Gelu)
            nc.scalar.activation(out=pw[:, 1:2], in_=pw[:, 1:2], func=AF.Sigmoid)

            jt = singles.tile([128, 640], BF16, name="jt")
            nc.vector.memset(jt[:, 0:128], 0.0)
            nc.vector.memset(jt[:, 128:640], 0.0)
            for wi in range(HEAD_MMS):
                wp = mmpsum.tile([128, 512], F32, tag="mm", name=f"hw{wi}")
                nc.tensor.matmul(
                    wp[:, :512], lhsT=jt[:, 0:128], rhs=jt[:, 128:640],
                    start=True, stop=True,
                )

            s_gate = singles.tile([128, CB], F32)   # sigmoid(l1-l0) = 1 - a0

            # xh: packed dense blocks, padded token layout, persistent.
            xh = xh_pool.tile([128, HB, XHSPAN], BF16)
            nc.vector.memset(xh[:, :, :GUARD], 0.0)
            nc.vector.memset(xh[:, :, GUARD + FRPAD:], 0.0)
            xh_rows = xh[:, :, GUARD:GUARD + FRPAD].rearrange(
                "p g (r c) -> p g r c", c=RP
            )
            nc.vector.memset(xh_rows[:, :, :, W:], 0.0)
            # gathered group-8 tile (copies bring zeroed pad cols with them)
            xh8 = xh_pool.tile([128, XHSPAN], BF16, name="xh8")
            nc.vector.memset(xh8[:, :GUARD], 0.0)
            nc.vector.memset(xh8[:, GUARD + FRPAD:], 0.0)

            hw_tiles = []

            def shifted_rhs(g, rg):
                sh, sw = SHIFTS[g]
                off = -(sh * RP + sw)
                s0 = GUARD + rg * RGP + off
                return xh[0:GS_HID, g, s0:s0 + RGP].rearrange(
                    "p (r c) -> p r c", c=RP
                )[:, :, :W]

            def shifted_rhs8(rg):
                sh, sw = SHIFTS[8]
                off = -(sh * RP + sw)
                s0 = GUARD + rg * RGP + off
                return xh8[0:G8H, s0:s0 + RGP].rearrange(
                    "p (r c) -> p r c", c=RP
                )[:, :, :W]

            for f in range(NF):
                h_t = h_pool.tile([128, CB, HWTOK], BF16, tag="h")
                nc.gpsimd.memset(h_t[:], 0.0)
                h4 = h_t.rearrange("p c (i j) -> p c i j", j=W)
                wsum_st = small.tile([128, CB, RG], F32, tag=f"wsst{f}")
                dsum_st = small.tile([128, CB, RG], F32, tag=f"dsst{f}")

                def h_evac(ps, rg, mb):
                    """Inverse-shift evacuation of one 128-row block: two
                    57-row main-group writes, plus the two 7-row g8 fragments
                    via a 32-aligned scratch activation (compute-engine APs
                    must start at 32-aligned partitions) whose g8 rows are
                    then DMA-copied into place."""
                    ps3 = ps[:, :RGT].rearrange("p (r c) -> p r c", c=W)
                    for half in range(2):
                        p0 = half * 64
                        g = 2 * mb + half
                        sh, sw = SHIFTS[g]
                        i0 = max(0, 8 * rg - sh)
                        i1 = min(H, 8 * rg + 8 - sh)
                        j0, j1 = max(0, -sw), min(W, W - sw)
                        nc.scalar.activation(
                            out=h4[p0:p0 + GS_C, mb, i0:i1, j0:j1],
                            in_=ps3[
                                p0:p0 + GS_C,
                                i0 + sh - 8 * rg:i1 + sh - 8 * rg,
                                j0 + sw:j1 + sw,
                            ],
                            func=AF.Gelu,
                            bias=fc1b_s[p0:p0 + GS_C, mb:mb + 1],
                        )
                    # g8 fragments via 32-row scratch activations (a pattern
                    # starting at partition 32k may span at most 32
                    # partitions, so one per half)
                    sh, sw = SHIFTS[8]
                    i0 = max(0, 8 * rg - sh)
                    i1 = min(H, 8 * rg + 8 - sh)
                    j0, j1 = max(0, -sw), min(W, W - sw)
                    scr = w_pool.tile([128, RGR + 1, W], BF16, tag="scr")
                    for half in range(2):
                        q0 = half * 64 + 32
                        nc.scalar.activation(
                            out=scr[q0:q0 + 32, 0:i1 - i0, j0:j1],
                            in_=ps3[
                                q0:q0 + 32,
                                i0 + sh - 8 * rg:i1 + sh - 8 * rg,
                                j0 + sw:j1 + sw,
                            ],
                            func=AF.Gelu,
                            bias=fc1b_s[q0:q0 + 32, mb:mb + 1],
                        )
                        f0 = half * 64 + GS_C
                        nc.sync.dma_start(
                            out=h4[f0:f0 + 7, mb, i0:i1, j0:j1],
                            in_=scr[f0:f0 + 7, 0:i1 - i0, j0:j1],
                        )

                prev_w = [None] * RG

                def d_spill(rg):
                    w_prev = prev_w[rg]
                    nc.vector.tensor_tensor(
                        w_prev[:],
                        h_t[:, :, rg * RGT:(rg + 1) * RGT],
                        w_prev[:],
                        ALU.subtract,
                    )
                    nc.sync.dma_start(
                        out=dsp[f][:, :, rg * RGT:(rg + 1) * RGT], in_=w_prev[:]
                    )
                    # gate-sum bookkeeping: sum(h) = sum(d) + sum(w)
                    nc.vector.tensor_reduce(
                        out=dsum_st[:, :, rg:rg + 1], in_=w_prev[:],
                        axis=mybir.AxisListType.X, op=ALU.add,
                    )

                def d_spill_mb(rg, mb):
                    """Per-block d = h - w spill: used for the last row group
                    so the AllReduce launch chain after the final matmul is
                    only one block's sub + reduce."""
                    w_prev = prev_w[rg]
                    nc.vector.tensor_tensor(
                        w_prev[:, mb],
                        h_t[:, mb, rg * RGT:(rg + 1) * RGT],
                        w_prev[:, mb],
                        ALU.subtract,
                    )
                    nc.sync.dma_start(
                        out=dsp[f][:, mb, rg * RGT:(rg + 1) * RGT],
                        in_=w_prev[:, mb],
                    )
                    nc.vector.tensor_reduce(
                        out=dsum_st[:, mb:mb + 1, rg:rg + 1], in_=w_prev[:, mb],
                        axis=mybir.AxisListType.X, op=ALU.add,
                    )

                def fc1_rg(rg, spill=False):
                    for mb in range(CB):
                        ps = mmpsum.tile([128, 512], F32, tag="mm")
                        for g in range(HB):
                            nc.tensor.matmul(
                                ps[:, :RGT],
                                lhsT=fc1wd_s[0:GS_HID, g, mb * 128:(mb + 1) * 128],
                                rhs=shifted_rhs(g, rg),
                                start=(g == 0),
                                stop=False,
                            )
                        nc.tensor.matmul(
                            ps[:, :RGT],
                            lhsT=fc1w8_s[0:G8H, mb * 128:(mb + 1) * 128],
                            rhs=shifted_rhs8(rg),
                            start=False,
                            stop=True,
                        )
                        h_evac(ps, rg, mb)
                        if spill:
                            # rg-1's spill must also wait for THIS rg's
                            # evacuation (shifted writes reach back one row)
                            d_spill_mb(rg, mb)
                            d_spill_mb(rg - 1, mb)

                for rg in range(RG):
                    # ---------- A: xh = gelu(x @ fc_w + fc_b), packed dense
                    xt_fetch(f, rg)
                    if rg + 1 < RG:
                        xt_fetch(f, rg + 1)
                    else:
                        xt_fetch(f + 1, 0)
                    xt_t = xts.pop((f, rg))
                    base = GUARD + rg * RGP
                    for mb in range(HB):
                        ps = mmpsum.tile([128, 512], F32, tag="mm")
                        for k in range(CCB):
                            nc.tensor.matmul(
                                ps[:, :RGT],
                                lhsT=fcw_s[:, k, mb * 128:(mb + 1) * 128],
                                rhs=xt_t[:, k, :],
                                start=(k == 0),
                                stop=(k == CCB - 1),
                            )
                        dst = xh[:, mb, base:base + RGP].rearrange(
                            "p (r c) -> p r c", c=RP
                        )[:, :, :W]
                        src = ps[:, :RGT].rearrange("p (r c) -> p r c", c=W)
                        nc.scalar.activation(
                            out=dst, in_=src, func=AF.Gelu,
                            bias=fcb_s[:, mb:mb + 1],
                        )
                        # gather this block's group-8 slice (contiguous span,
                        # pad cols are zero in both src and dst)
                        nc.sync.dma_start(
                            out=xh8[G8B * mb:G8B * (mb + 1), base:base + RGP],
                            in_=xh[GS_HID:128, mb, base:base + RGP],
                        )

                    # ---------- B: w = gelu(xh @ fc2_w + b), dense contraction
                    rhs_pl = xh[:, :, base:base + RGP].rearrange(
                        "p g (r c) -> p g r c", c=RP)[:, :, :, :W]
                    w_rg = w_pool.tile([128, CB, RGT], BF16, tag="wrg", bufs=5)
                    for mb in range(CB):
                        ps = mmpsum.tile([128, 512], F32, tag="mm")
                        for kb in range(HB):
                            nc.tensor.matmul(
                                ps[:, :RGT],
                                lhsT=fc2w_s[:, kb, mb * 128:(mb + 1) * 128],
                                rhs=rhs_pl[:, kb],
                                start=(kb == 0),
                                stop=(kb == HB - 1),
                            )
                        dst = w_rg[:, mb, :].rearrange("p (r c) -> p r c", c=W)
                        srcp = ps[:, :RGT].rearrange("p (r c) -> p r c", c=W)
                        nc.scalar.activation(
                            out=dst, in_=srcp, func=AF.Gelu,
                            bias=fc2b_s[:, mb:mb + 1],
                        )
                    nc.vector.tensor_reduce(
                        out=wsum_st[:, :, rg:rg + 1], in_=w_rg[:],
                        axis=mybir.AxisListType.X, op=ALU.add,
                    )
                    prev_w[rg] = w_rg

                    if f == 0 and rg == 0:
                        load_rest()
                        projw_s = _rest["projw_s"]; projb_s = _rest["projb_s"]
                        rw1w_s = _rest["rw1w_s"]; rw1b_s = _rest["rw1b_s"]
                        rwdw_s = _rest["rwdw_s"]; rwdb_s = _rest["rwdb_s"]

                    # ---------- C: h (lag 2 so the group-8 gather copies
                    # always have a full row-group step of slack)
                    if rg >= 2:
                        fc1_rg(rg - 2)
                        if rg >= 3:
                            d_spill(rg - 3)
                # gate partial sums: everything except the last row-group's
                # per-block d-sums is folded into `pbase` during fc1(6), so
                # the AllReduce launch chain after the final matmul is one
                # block's sub + reduce + add + the ccin DMA (on the idle
                # ScalarE HWDGE queue to skip the sync FIFO).
                fc1_rg(RG - 2)
                d_spill(RG - 3)
                ws = small.tile([128, CB], F32, tag=f"ws{f}")
                nc.vector.tensor_reduce(
                    out=ws, in_=wsum_st[:], axis=mybir.AxisListType.X, op=ALU.add
                )
                dsm = small.tile([128, CB], F32, tag=f"dsm{f}")
                nc.vector.tensor_reduce(
                    out=dsm, in_=dsum_st[:, :, 0:RG - 2],
                    axis=mybir.AxisListType.X, op=ALU.add,
                )
                pbase = small.tile([128, CB], F32, tag=f"pb{f}")
                nc.vector.tensor_scalar_mul(pbase, ws, 2.0)
                nc.vector.tensor_tensor(pbase, pbase, dsm, ALU.add)
                fc1_rg(RG - 1, spill=True)

                # part = sum(h) + sum(w) = sum(d) + 2*sum(w)
                part = small.tile([128, CB], F32, tag=f"part{f}")
                nc.vector.tensor_tensor(
                    part, pbase,
                    dsum_st[:, :, RG - 2:RG - 1].rearrange("p c o -> p (c o)"),
                    ALU.add,
                )
                nc.vector.tensor_tensor(
                    part, part,
                    dsum_st[:, :, RG - 1:RG].rearrange("p c o -> p (c o)"),
                    ALU.add,
                )
                nc.scalar.dma_start(out=ccin[f][:], in_=part)
                nc.gpsimd.collective_compute(
                    "AllReduce",
                    ALU.add,
                    replica_groups=GROUPS,
                    ins=[ccin[f][:]],
                    outs=[ccout[f][:]],
                )

                hw_tiles.append(h_t)

            # bridge the second AllReduce's latency window (collective floor
            # + cross-core launch skew) with junk matmuls so the PE activity
            # monitor keeps the full clock.
            for wi in range(WARM_MMS):
                wp = mmpsum.tile([128, 512], F32, tag="mm", name=f"warm{wi}")
                nc.tensor.matmul(
                    wp[:, :512],
                    lhsT=fcw_s[:, 0, 0:128],
                    rhs=fcw_s[:, 1, 0:512],
                    start=True,
                    stop=True,
                )

            # ---------------- gate: s = sigmoid(l1 - l0) = 1 - a0
            # (1/MEAN_N folded into rw1w, l1-l0 weights precomputed on host)
            za = small.tile([128, 2 * CB], F32, tag="za")
            nc.sync.dma_start(out=za[:, 0:CB], in_=ccout[0][:])
            nc.sync.dma_start(out=za[:, CB:2 * CB], in_=ccout[1][:])
            zbf = small.tile([128, CB], BF16, tag="zbf")
            nc.vector.tensor_tensor(zbf, za[:, 0:CB], za[:, CB:2 * CB], ALU.add)

            psg = mmpsum.tile([128, 512], F32, tag="mm", name="psg")[:, :1]
            for k in range(CB):
                nc.tensor.matmul(
                    psg,
                    lhsT=rw1w_s[:, k, :],
                    rhs=zbf[:, k:k + 1],
                    start=(k == 0),
                    stop=(k == CB - 1),
                )
            gv = small.tile([128, 1], BF16, tag="gv")
            nc.scalar.activation(out=gv, in_=psg, func=AF.Gelu, bias=rw1b_s[:, 0:1])
            psu = mmpsum.tile([128, 512], F32, tag="mm", name="psu")[:, :CB]
            for m in range(CB):
                nc.tensor.matmul(
                    psu[:, m:m + 1],
                    lhsT=rwdw_s[:, m * 128:(m + 1) * 128],
                    rhs=gv,
                    start=True,
                    stop=True,
                )
            ldif = small.tile([128, CB], F32, tag="ldif")
            nc.vector.tensor_tensor(ldif, psu, rwdb_s, ALU.add)
            nc.scalar.activation(out=s_gate, in_=ldif, func=AF.Sigmoid)

            # ---------------- D: out = (h - s*d) @ proj_w + proj_b
            def emit_out(pp, fidx, t0, M):
                ot = ostage.tile([128, C], BF16, tag="ot")
                nc.vector.tensor_tensor(ot[:M], pp[:M, :C], projb_s[:M], ALU.add)
                nc.sync.dma_start(out=out_d[fidx, t0:t0 + M, :], in_=ot[:M])

            # h is resident for both frames; stream each frame's d back in
            # DCK-token chunks, gate into the d tile, then project.  The two
            # frames' chunks are interleaved so their independent
            # DMA->gate->matmul chains hide each other's latency; scale and
            # subtract alternate per k-block (ScalarE / DVE) so the first
            # proj matmul starts as soon as k-block 0 is gated.
            chunk_starts = [HWTOK - HWTOK % DCK] + list(range(0, HWTOK - HWTOK % DCK, DCK))
            for ck0 in chunk_starts:
                for fidx in (0, 1):
                    h_t = hw_tiles[fidx]
                    CK = min(DCK, HWTOK - ck0)
                    dc = dstream.tile([128, CB, DCK], BF16, tag="wc")
                    nc.sync.dma_start(
                        out=dc[:, :, :CK], in_=dsp[fidx][:, :, ck0:ck0 + CK]
                    )
                    for kb in range(CB):
                        if kb < 3:
                            nc.scalar.activation(
                                out=dc[:, kb, :CK], in_=dc[:, kb, :CK],
                                func=AF.Copy, scale=s_gate[:, kb:kb + 1],
                            )
                        else:
                            nc.vector.tensor_scalar_mul(
                                dc[:, kb, :CK], dc[:, kb, :CK],
                                s_gate[:, kb:kb + 1],
                            )
                        nc.vector.tensor_tensor(
                            dc[:, kb, :CK],
                            h_t[:, kb, ck0:ck0 + CK],
                            dc[:, kb, :CK],
                            ALU.subtract,
                        )
                    m0 = 0
                    while m0 < CK:
                        M = min(128, CK - m0)
                        pp = mmpsum.tile([128, 512], F32, tag="mm")
                        for kb in range(CB):
                            nc.tensor.matmul(
                                pp[:M, :C],
                                lhsT=dc[:, kb, m0:m0 + M],
                                rhs=projw_s[:, kb, :],
                                start=(kb == 0),
                                stop=(kb == CB - 1),
                            )
                        emit_out(pp, fidx, ck0 + m0, M)
                        m0 += M

    nc.compile()
    return nc


# ---------------------------------------------------------------- host side
def _prep_weights(fc_w, fc_b, fc1_w, fc1_b, fc2_w, fc2_b,
                  rw1_w, rw1_b, rw2_w, rw2_b, proj_w, proj_b):
    f32 = np.float32
    qof = _qof()
    hp = _hperm()

    # fc: columns in packed dense hid order, 4 k-blocks of x on partitions
    fcw_h = np.ascontiguousarray(
        fc_w[:, hp].reshape(CCB, 128, HID).transpose(1, 0, 2)
    ).astype(BF16_NP)
    fcb_h = np.ascontiguousarray(fc_b[hp].reshape(HB, 128).T).astype(f32)

    # fc1/fc2: rows in packed dense order, columns in interleaved-512 order
    fc1wd_h = np.ascontiguousarray(
        fc1_w[hp][:, qof].reshape(HB, 128, C).transpose(1, 0, 2)
    ).astype(BF16_NP)
    fc2w_h = np.ascontiguousarray(
        fc2_w[hp][:, qof].reshape(HB, 128, C).transpose(1, 0, 2)
    ).astype(BF16_NP)
    w8 = np.zeros((128, C), f32)
    w8[:G8H] = fc1_w[8 * GS_HID:][:, qof]
    fc1w8_h = np.ascontiguousarray(w8).astype(BF16_NP)

    fc1b_h = np.ascontiguousarray(fc1_b[qof].reshape(CB, 128).T).astype(f32)
    fc2b_h = np.ascontiguousarray(fc2_b[qof].reshape(CB, 128).T).astype(f32)

    projw_h = np.ascontiguousarray(
        proj_w[qof].reshape(CB, 128, C).transpose(1, 0, 2)
    ).astype(BF16_NP)
    projb_h = np.ascontiguousarray(
        np.broadcast_to(proj_b[None, :], (128, C))
    ).astype(f32)

    rw1w_h = np.ascontiguousarray(
        (rw1_w / MEAN_N)[qof].reshape(CB, 128, C // 4).transpose(1, 0, 2)
    ).astype(BF16_NP)
    rw1b_h = np.ascontiguousarray(rw1_b[:, None]).astype(f32)

    # gate difference weights: s = sigmoid(l1 - l0) = 1 - a0, so
    # gated = h*a0 + w*(1-a0) = h - s*(h-w)
    rwdw_h = np.ascontiguousarray(
        rw2_w[:, 2 * qof + 1] - rw2_w[:, 2 * qof]
    ).astype(BF16_NP)
    rwdb_h = np.ascontiguousarray(
        (rw2_b[2 * qof + 1] - rw2_b[2 * qof]).reshape(CB, 128).T
    ).astype(f32)

    return dict(
        fcw=fcw_h, fcb=fcb_h, fc1wd=fc1wd_h, fc1w8=fc1w8_h, fc1b=fc1b_h,
        fc2w=fc2w_h, fc2b=fc2b_h, projw=projw_h, projb=projb_h,
        rw1w=rw1w_h, rw1b=rw1b_h, rwdw=rwdw_h, rwdb=rwdb_h,
    )


def _get_nc():
    if "nc" not in _CACHE:
        _CACHE["nc"] = build_nc()
    return _CACHE["nc"]


def run(inputs, trace=False, trace_kwargs=None):
    """Run the SPMD kernel; returns (full_output, BassKernelResults)."""
    x = np.asarray(inputs["x"], np.float32)
    shared = _prep_weights(
        np.asarray(inputs["fc_w"], np.float32), np.asarray(inputs["fc_b"], np.float32),
        np.asarray(inputs["fc1_w"], np.float32), np.asarray(inputs["fc1_b"], np.float32),
        np.asarray(inputs["fc2_w"], np.float32), np.asarray(inputs["fc2_b"], np.float32),
        np.asarray(inputs["rw1_w"], np.float32), np.asarray(inputs["rw1_b"], np.float32),
        np.asarray(inputs["rw2_w"], np.float32), np.asarray(inputs["rw2_b"], np.float32),
        np.asarray(inputs["proj_w"], np.float32), np.asarray(inputs["proj_b"], np.float32),
    )

    xf = x.reshape(B * T, HWTOK, C)
    in_maps = []
    for c in range(NCORES):
        sh = xf[NF * c:NF * (c + 1)]                      # [NF, 3136, 512]
        xt = sh.transpose(0, 2, 1).reshape(NF, CCB, 128, HWTOK)
        xt = np.ascontiguousarray(xt.transpose(0, 2, 1, 3)).astype(BF16_NP)
        m = dict(shared)
        m["xT"] = xt
        in_maps.append(m)

    nc = _get_nc()
    res = run_bass_kernel_spmd(
        nc, in_maps, list(range(NCORES)),
        trace=trace, **(dict(trace_kwargs=trace_kwargs) if trace_kwargs else {}),
    )

    out = np.empty((B * T, HWTOK, C), np.float32)
    for c in range(NCORES):
        out[NF * c:NF * (c + 1)] = np.asarray(
            res.results[c]["out"], dtype=np.float32
        )
    return out.reshape(B, T, H, W, C), res


def kernel(**inputs) -> np.ndarray:
    full, _ = run(inputs, trace=False)
    return full
